# revision 10
# baseline (speedup 1.0000x reference)
"""Trainium2 Bass kernel for nn_DoubleNet (segment_reduce).

Computation (per reference):
  weight branch : BN(eval) -> Linear(D,H) -> ReLU -> Linear(H,1) -> Sigmoid   [N,1]
  quality branch: Linear(D,H) -> ReLU -> Linear(H,BINS), scaled by weight     [N,BINS]
  segment mean over uniform 50-row segments -> [T,BINS] -> softmax / NLL loss

Device strategy (8 cores, data-parallel over rows):
  - Host pre-transposes each core's data shard to feature-major xT [D, rows]
    and folds the BatchNorm affine into the weight-branch Linear.
  - Feature-major pipeline per 500-row block:
      mm1 (f32r matmuls, K=256 in 2 chunks)  -> psum [128,500] per h-chunk
      ReLU+bias on the psum->sbuf copy (ACT for quality, DVE for weight branch)
      mm2-q: lhsT = qw2.T (padded M=32), col-tiled 4 blocks per psum tile
      mm2-w: lhsT = ww2 replicated across M -> column-sum trick: psum gets the
             sigmoid argument replicated; ACT applies Sigmoid(+wb2)
      DVE: (q2 + qb2) * w  fused via scalar_tensor_tensor, then a strided
           segment reduce [128,10,50] -> [128,10] into a packed accumulator.
  - Device output per core: packed segment sums [128, 320] (f32).
  - Host: unpack, /50, softmax, loss (tiny [T,16] math).
"""

import os
import sys

sys.path.insert(0, "/opt/trn_rl_repo")

import numpy as np  # noqa: E402

N_TOTAL = 500000
D = 256
H = 256
BINS = 16
T_SEGS = 10000
SEG = 50
EPS = 1e-5

N_CORES = 8
RPC = N_TOTAL // N_CORES          # 62500 rows per core
BLK = 500                         # rows per matmul block (10 segments)
SEGS_PER_BLK = BLK // SEG         # 10
BLKS_PER_GROUP = 4                # packed into one psum tile via col-tiling

_PROGRAM_CACHE = {}


def _groups_for(rpc):
    """List of (group_index, n_blocks) covering rpc rows."""
    nblk = rpc // BLK
    assert nblk * BLK == rpc
    groups = []
    g = 0
    rem = nblk
    while rem > 0:
        nb = min(BLKS_PER_GROUP, rem)
        groups.append((g, nb))
        g += 1
        rem -= nb
    return groups


def _build_program(rpc):
    """Build the Bass program for one core processing `rpc` rows."""
    import concourse.bass as bass
    import concourse.tile as tile
    from concourse import bacc, mybir

    dt = mybir.dt
    Act = mybir.ActivationFunctionType
    Alu = mybir.AluOpType

    groups = _groups_for(rpc)
    n_groups = len(groups)
    s_cols = n_groups * SEGS_PER_BLK

    nc = bacc.Bacc("TRN2", target_bir_lowering=False, debug=False)

    # ---- I/O ----
    xT = nc.dram_tensor("xT", [D, rpc], dt.float32r, kind="ExternalInput")
    w1qT_d = nc.dram_tensor("w1qT", [D, H], dt.float32r, kind="ExternalInput")
    w1wT_d = nc.dram_tensor("w1wT", [D, H], dt.float32r, kind="ExternalInput")
    qb1_d = nc.dram_tensor("qb1", [128, 2], dt.float32, kind="ExternalInput")
    wb1_d = nc.dram_tensor("wb1", [128, 2], dt.float32, kind="ExternalInput")
    # mm2 lhsT variants: variant j has its 16 (or 32) live columns at
    # partition-strip [32j, 32j+32) and zeros elsewhere, so four blocks'
    # mm2 matmuls can accumulate into one shared [128, BLK] psum tile with
    # block j's outputs landing on partitions [32j, 32j+32).
    w2qP_d = nc.dram_tensor("w2qP", [H, 512], dt.bfloat16, kind="ExternalInput")
    ww2P_d = nc.dram_tensor("ww2P", [H, 512], dt.bfloat16, kind="ExternalInput")
    qb2_d = nc.dram_tensor("qb2", [128, 1], dt.float32, kind="ExternalInput")
    wb2_d = nc.dram_tensor("wb2", [128, 1], dt.float32, kind="ExternalInput")
    s_out = nc.dram_tensor("S_out", [128, s_cols], dt.float32, kind="ExternalOutput")

    with tile.TileContext(nc) as tc:
        with (
            tc.tile_pool(name="wpool", bufs=1) as wpool,
            tc.tile_pool(name="xpool", bufs=6) as xpool,
            tc.tile_pool(name="hq", bufs=10) as hq_pool,
            tc.tile_pool(name="hw", bufs=10) as hw_pool,
            tc.tile_pool(name="misc", bufs=3) as misc_pool,
            tc.tile_pool(name="sall", bufs=1) as sall_pool,
            tc.tile_pool(name="ps1", bufs=4, space=bass.MemorySpace.PSUM) as ps1,
            tc.tile_pool(name="ps2", bufs=2, space=bass.MemorySpace.PSUM) as ps2,
        ):
            # ---- load weights/constants into SBUF ----
            w1q_sb = []
            w1w_sb = []
            w2q_sb = []
            ww2_sb = []
            for kc in range(2):
                t = wpool.tile([128, H], dt.float32r, tag=f"w1q{kc}")
                nc.sync.dma_start(t[:, :], xT_sl(w1qT_d, kc))
                w1q_sb.append(t)
                t = wpool.tile([128, H], dt.float32r, tag=f"w1w{kc}")
                nc.sync.dma_start(t[:, :], xT_sl(w1wT_d, kc))
                w1w_sb.append(t)
                w2q_row = []
                ww2_row = []
                for j in range(4):
                    t = wpool.tile([128, 128], dt.bfloat16, tag=f"w2q{kc}_{j}",
                                   name=f"w2q{kc}_{j}")
                    nc.sync.dma_start(
                        t[:, :], w2qP_d[kc * 128:(kc + 1) * 128,
                                        128 * j:128 * (j + 1)])
                    w2q_row.append(t)
                    t = wpool.tile([128, 128], dt.bfloat16, tag=f"ww2{kc}_{j}",
                                   name=f"ww2{kc}_{j}")
                    nc.sync.dma_start(
                        t[:, :], ww2P_d[kc * 128:(kc + 1) * 128,
                                        128 * j:128 * (j + 1)])
                    ww2_row.append(t)
                w2q_sb.append(w2q_row)
                ww2_sb.append(ww2_row)
            qb1_sb = wpool.tile([128, 2], dt.float32, tag="qb1")
            nc.sync.dma_start(qb1_sb[:, :], qb1_d[:, :])
            wb1_sb = wpool.tile([128, 2], dt.float32, tag="wb1")
            nc.sync.dma_start(wb1_sb[:, :], wb1_d[:, :])
            qb2_sb = wpool.tile([128, 1], dt.float32, tag="qb2")
            nc.sync.dma_start(qb2_sb[:, :], qb2_d[:, :])
            wb2_sb = wpool.tile([128, 1], dt.float32, tag="wb2")
            nc.sync.dma_start(wb2_sb[:, :], wb2_d[:, :])

            s_all = sall_pool.tile([128, s_cols], dt.float32, tag="sall")
            nc.vector.memset(s_all[:, :], 0.0)

            for g, nb in groups:
                psum_q2 = ps2.tile([128, BLK], dt.float32, tag="q2")
                psum_sig = ps2.tile([128, BLK], dt.float32, tag="sig")
                for j in range(nb):
                    blk = g * BLKS_PER_GROUP + j
                    c0 = blk * BLK
                    # load x.T block (feature-major), 2 partition chunks
                    xt = [xpool.tile([128, BLK], dt.float32r, tag="x",
                                     name=f"x{blk}_{kc2}") for kc2 in range(2)]
                    for kc in range(2):
                        nc.sync.dma_start(
                            xt[kc][:, :], xT[kc * 128:(kc + 1) * 128, c0:c0 + BLK]
                        )
                    qh = []
                    wh = []
                    for hc in range(2):
                        # quality branch mm1 + ReLU(+qb1) via ACT
                        psq = ps1.tile([128, BLK], dt.float32, tag="mm1")
                        for kc in range(2):
                            nc.tensor.matmul(
                                psq[:, :],
                                w1q_sb[kc][:, hc * 128:(hc + 1) * 128],
                                xt[kc][:, :],
                                start=(kc == 0),
                                stop=(kc == 1),
                            )
                        qh_t = hq_pool.tile([128, BLK], dt.bfloat16, tag="qh")
                        nc.scalar.activation(
                            qh_t[:, :], psq[:, :], Act.Relu,
                            bias=qb1_sb[:, hc:hc + 1], scale=1.0,
                        )
                        qh.append(qh_t)
                        # weight branch mm1 (BN folded) + ReLU(+wb1') via DVE
                        psw = ps1.tile([128, BLK], dt.float32, tag="mm1")
                        for kc in range(2):
                            nc.tensor.matmul(
                                psw[:, :],
                                w1w_sb[kc][:, hc * 128:(hc + 1) * 128],
                                xt[kc][:, :],
                                start=(kc == 0),
                                stop=(kc == 1),
                            )
                        wh_t = hw_pool.tile([128, BLK], dt.bfloat16, tag="wh")
                        nc.vector.tensor_scalar(
                            wh_t[:, :], psw[:, :],
                            wb1_sb[:, hc:hc + 1], 0.0,
                            op0=Alu.add, op1=Alu.max,
                        )
                        wh.append(wh_t)
                    # mm2: zero-padded lhsT variants accumulate block j's
                    # outputs onto partitions [32j, 32j+32) of shared psum
                    for kc in range(2):
                        nc.tensor.matmul(
                            psum_q2[:, :], w2q_sb[kc][j][:, :], qh[kc][:, :],
                            start=(j == 0 and kc == 0),
                            stop=(j == nb - 1 and kc == 1),
                        )
                    for kc in range(2):
                        nc.tensor.matmul(
                            psum_sig[:, :], ww2_sb[kc][j][:, :], wh[kc][:, :],
                            start=(j == 0 and kc == 0),
                            stop=(j == nb - 1 and kc == 1),
                        )
                p = 128
                # w = sigmoid(colsum + wb2), replicated over each 32-row strip
                w_t = misc_pool.tile([128, BLK], dt.bfloat16, tag="w")
                nc.scalar.activation(
                    w_t[:p, :], psum_sig[:p, :], Act.Sigmoid,
                    bias=wb2_sb[:p, 0:1], scale=1.0,
                )
                # t2 = (q2 + qb2) * w
                t2 = misc_pool.tile([128, BLK], dt.bfloat16, tag="t2")
                nc.vector.scalar_tensor_tensor(
                    t2[:p, :], psum_q2[:p, :], qb2_sb[:p, 0:1], w_t[:p, :],
                    op0=Alu.add, op1=Alu.mult,
                )
                # segment sums: [p, 10, 50] -> [p, 10]
                c = g * SEGS_PER_BLK
                nc.vector.tensor_reduce(
                    s_all[:p, c:c + SEGS_PER_BLK],
                    t2[:p, :].rearrange("p (s i) -> p s i", i=SEG),
                    axis=mybir.AxisListType.X,
                    op=Alu.add,
                )

            nc.sync.dma_start(s_out[:, :], s_all[:, :])

    nc.compile()
    return nc


def xT_sl(dram, kc):
    """Rows [kc*128, (kc+1)*128) of a DRAM tensor."""
    return dram[kc * 128:(kc + 1) * 128, :]


def _host_weights(qw1, qb1, qw2, qb2, bn_gamma, bn_beta, bn_mean, bn_var,
                  ww1, wb1, ww2, wb2):
    """Fold BN, transpose weights into lhsT layouts, pack biases."""
    import ml_dtypes

    f32 = np.float32
    s = (bn_gamma / np.sqrt(bn_var + EPS)).astype(f32)
    bpp = (bn_beta - bn_mean * s).astype(f32)
    w1qT = np.ascontiguousarray(qw1.T.astype(f32))                   # [D,H]
    w1wT = np.ascontiguousarray((ww1 * s[None, :]).T.astype(f32))    # [D,H]
    b1w = (ww1.astype(np.float64) @ bpp.astype(np.float64)).astype(f32) + wb1.astype(f32)

    qb1_p = np.ascontiguousarray(qb1.astype(f32).reshape(2, 128).T)  # [128,2]
    wb1_p = np.ascontiguousarray(b1w.reshape(2, 128).T)              # [128,2]

    w2qP = np.zeros((H, 512), dtype=ml_dtypes.bfloat16)
    ww2P = np.zeros((H, 512), dtype=ml_dtypes.bfloat16)
    for j in range(4):
        w2qP[:, 128 * j + 32 * j:128 * j + 32 * j + BINS] = \
            qw2.T.astype(ml_dtypes.bfloat16)
        ww2P[:, 128 * j + 32 * j:128 * j + 32 * j + 32] = \
            np.repeat(ww2.reshape(H, 1).astype(ml_dtypes.bfloat16), 32, axis=1)

    qb2_p = np.zeros((128, 1), dtype=f32)
    for j in range(4):
        qb2_p[32 * j:32 * j + BINS, 0] = qb2.astype(f32)
    wb2_p = np.full((128, 1), np.float32(wb2.reshape(-1)[0]), dtype=f32)

    return dict(w1qT=w1qT, w1wT=w1wT, qb1=qb1_p, wb1=wb1_p,
                w2qP=w2qP, ww2P=ww2P, qb2=qb2_p, wb2=wb2_p)


def _unpack_sums(s_packed, rpc):
    """[128, s_cols] packed -> [rpc//SEG, BINS] segment sums."""
    n_groups = len(_groups_for(rpc))
    segs = rpc // SEG
    arr = s_packed.reshape(4, 32, n_groups, SEGS_PER_BLK)   # [j, b, g, i]
    out = arr.transpose(2, 0, 3, 1).reshape(n_groups * 4 * SEGS_PER_BLK, 32)
    return out[:segs, :BINS]


def _numpy_fallback(data, qw1, qb1, qw2, qb2, bn_gamma, bn_beta, bn_mean,
                    bn_var, ww1, wb1, ww2, wb2, tag, separates):
    """Pure-host reference path (used only for non-uniform segments)."""
    f = np.float32
    x = data.astype(f)
    s = (bn_gamma / np.sqrt(bn_var + EPS)).astype(f)
    xn = (x - bn_mean.astype(f)) * s + bn_beta.astype(f)
    wh = np.maximum(xn @ ww1.T.astype(f) + wb1.astype(f), 0)
    wout = 1.0 / (1.0 + np.exp(-(wh @ ww2.T.astype(f) + wb2.astype(f))))
    qh = np.maximum(x @ qw1.T.astype(f) + qb1.astype(f), 0)
    q = (qh @ qw2.T.astype(f) + qb2.astype(f)) * wout
    sep = np.asarray(separates).astype(np.int64)
    sums = np.add.reduceat(q, sep[:-1], axis=0)
    lengths = (sep[1:] - sep[:-1]).astype(np.float64)
    return sums, lengths


def _finish(sums, lengths, tag):
    """sums [T,BINS], lengths [T] -> (out_probs f32 [T,BINS], loss f32)."""
    logits = sums.astype(np.float64) / lengths[:, None]
    m = logits.max(axis=1, keepdims=True)
    e = np.exp(logits - m)
    z = e.sum(axis=1, keepdims=True)
    probs = e / z
    logp = (logits - m) - np.log(z)
    t = np.asarray(tag).astype(np.int64).reshape(-1)
    loss = -np.mean(logp[np.arange(logp.shape[0]), t])
    return probs.astype(np.float32), np.float32(loss)


def _run_device(data, shared, trace=False):
    from concourse.bass_utils import run_bass_kernel_spmd

    if RPC not in _PROGRAM_CACHE:
        _PROGRAM_CACHE[RPC] = _build_program(RPC)
    nc = _PROGRAM_CACHE[RPC]

    in_maps = []
    for i in range(N_CORES):
        shard = data[i * RPC:(i + 1) * RPC]
        xT = np.ascontiguousarray(shard.T.astype(np.float32))
        m = {"xT": xT}
        m.update(shared)
        in_maps.append(m)

    res = run_bass_kernel_spmd(nc, in_maps, list(range(N_CORES)), trace=trace)
    sums = np.concatenate(
        [_unpack_sums(res.results[i]["S_out"].astype(np.float64), RPC)
         for i in range(N_CORES)], axis=0)
    return sums, res


def _bench_device(data, shared, iters=5):
    """Steady-state timing of the NEFF execution via PJRT.

    Mirrors bass2jax.run_bass_via_pjrt but keeps the jitted callable and
    device-resident inputs so repeated calls time only the on-device
    execution (plus PJRT dispatch). Returns (results_list, times_sec).
    """
    import time

    import jax
    from jax.experimental.shard_map import shard_map
    from jax.sharding import Mesh, PartitionSpec

    from concourse import bass2jax, mybir

    if RPC not in _PROGRAM_CACHE:
        _PROGRAM_CACHE[RPC] = _build_program(RPC)
    nc = _PROGRAM_CACHE[RPC]
    bass2jax.install_neuronx_cc_hook()

    in_maps = []
    for i in range(N_CORES):
        shard = data[i * RPC:(i + 1) * RPC]
        xT = np.ascontiguousarray(shard.T.astype(np.float32))
        m = {"xT": xT}
        m.update(shared)
        in_maps.append(m)

    partition_name = (nc.partition_id_tensor.name
                      if nc.partition_id_tensor else None)
    in_names, out_names, out_avals, zero_outs = [], [], [], []
    for alloc in nc.m.functions[0].allocations:
        if not isinstance(alloc, mybir.MemoryLocationSet):
            continue
        name = alloc.memorylocations[0].name
        if alloc.kind == "ExternalInput":
            if name != partition_name:
                in_names.append(name)
        elif alloc.kind == "ExternalOutput":
            shape = tuple(alloc.tensor_shape)
            dtype = mybir.dt.np(alloc.dtype)
            out_names.append(name)
            out_avals.append(jax.core.ShapedArray(shape, dtype))
            zero_outs.append(np.zeros(shape, dtype))
    n_params = len(in_names)
    n_outs = len(out_avals)
    all_names = in_names + out_names
    if partition_name is not None:
        all_names = all_names + [partition_name]

    def _body(*args):
        operands = list(args)
        if partition_name is not None:
            operands.append(bass2jax.partition_id_tensor())
        outs = bass2jax._bass_exec_p.bind(
            *operands,
            out_avals=tuple(out_avals),
            in_names=tuple(all_names),
            out_names=tuple(out_names),
            lowering_input_output_aliases=(),
            sim_require_finite=True,
            sim_require_nnan=True,
            nc=nc,
        )
        return tuple(outs)

    devices = jax.devices()[:N_CORES]
    mesh = Mesh(np.asarray(devices), ("core",))
    in_specs = (PartitionSpec("core"),) * (n_params + n_outs)
    out_specs = (PartitionSpec("core"),) * n_outs
    sharded = jax.jit(
        shard_map(_body, mesh=mesh, in_specs=in_specs, out_specs=out_specs,
                  check_rep=False),
        keep_unused=True,
    )
    concat_in = [
        np.concatenate([np.asarray(in_maps[c][nm]) for c in range(N_CORES)], axis=0)
        for nm in in_names
    ]
    concat_zeros = [
        np.zeros((N_CORES * z.shape[0], *z.shape[1:]), z.dtype) for z in zero_outs
    ]
    dev_in = [jax.device_put(a) for a in concat_in]
    dev_zeros = [jax.device_put(a) for a in concat_zeros]

    out_arrs = sharded(*dev_in, *dev_zeros)  # warm-up + result
    jax.block_until_ready(out_arrs)
    times = []
    for _ in range(iters):
        t0 = time.perf_counter()
        o = sharded(*dev_in, *dev_zeros)
        jax.block_until_ready(o)
        times.append(time.perf_counter() - t0)

    results = [
        {nm: np.asarray(out_arrs[i]).reshape(N_CORES, *out_avals[i].shape)[c]
         for i, nm in enumerate(out_names)}
        for c in range(N_CORES)
    ]
    return results, times


def kernel(data, qw1, qb1, qw2, qb2, bn_gamma, bn_beta, bn_mean, bn_var,
           ww1, wb1, ww2, wb2, tag, separates, _trace=False, _return_res=False):
    data = np.asarray(data)
    sep = np.asarray(separates).astype(np.int64)
    args = [np.asarray(a) for a in (qw1, qb1, qw2, qb2, bn_gamma, bn_beta,
                                    bn_mean, bn_var, ww1, wb1, ww2, wb2)]
    uniform = (
        data.shape == (N_TOTAL, D)
        and sep.shape[0] == T_SEGS + 1
        and np.all(np.diff(sep) == SEG)
        and sep[0] == 0
    )
    if not uniform:
        sums, lengths = _numpy_fallback(data, *args, tag, sep)
        return _finish(sums, lengths, tag)

    shared = _host_weights(*args)
    sums, res = _run_device(data, shared, trace=_trace)
    lengths = np.full(T_SEGS, float(SEG))
    out = _finish(sums, lengths, tag)
    if _return_res:
        return out, res
    return out


# revision 11
# speedup vs baseline: 1.5532x; 1.5532x over previous
"""Trainium2 Bass kernel for nn_DoubleNet (segment_reduce).

Computation (per reference):
  weight branch : BN(eval) -> Linear(D,H) -> ReLU -> Linear(H,1) -> Sigmoid   [N,1]
  quality branch: Linear(D,H) -> ReLU -> Linear(H,BINS), scaled by weight     [N,BINS]
  segment mean over uniform 50-row segments -> [T,BINS] -> softmax / NLL loss

Device strategy (8 cores, data-parallel over rows):
  - Host pre-transposes each core's data shard to feature-major xT [D, rows]
    and folds the BatchNorm affine into the weight-branch Linear.
  - Feature-major pipeline per 500-row block:
      mm1 (f32r matmuls, K=256 in 2 chunks)  -> psum [128,500] per h-chunk
      ReLU+bias on the psum->sbuf copy (ACT for quality, DVE for weight branch)
      mm2-q: lhsT = qw2.T (padded M=32), col-tiled 4 blocks per psum tile
      mm2-w: lhsT = ww2 replicated across M -> column-sum trick: psum gets the
             sigmoid argument replicated; ACT applies Sigmoid(+wb2)
      DVE: (q2 + qb2) * w  fused via scalar_tensor_tensor, then a strided
           segment reduce [128,10,50] -> [128,10] into a packed accumulator.
  - Device output per core: packed segment sums [128, 320] (f32).
  - Host: unpack, /50, softmax, loss (tiny [T,16] math).
"""

import os
import sys

sys.path.insert(0, "/opt/trn_rl_repo")

import numpy as np  # noqa: E402

N_TOTAL = 500000
D = 256
H = 256
BINS = 16
T_SEGS = 10000
SEG = 50
EPS = 1e-5

N_CORES = 8
RPC = N_TOTAL // N_CORES          # 62500 rows per core
BLK = 500                         # rows per matmul block (10 segments)
SEGS_PER_BLK = BLK // SEG         # 10
BLKS_PER_GROUP = 4                # packed into one psum tile via col-tiling

_PROGRAM_CACHE = {}


def _groups_for(rpc):
    """List of (group_index, n_blocks) covering rpc rows."""
    nblk = rpc // BLK
    assert nblk * BLK == rpc
    groups = []
    g = 0
    rem = nblk
    while rem > 0:
        nb = min(BLKS_PER_GROUP, rem)
        groups.append((g, nb))
        g += 1
        rem -= nb
    return groups


def _build_program(rpc):
    """Build the Bass program for one core processing `rpc` rows."""
    import concourse.bass as bass
    import concourse.tile as tile
    from concourse import bacc, mybir

    dt = mybir.dt
    Act = mybir.ActivationFunctionType
    Alu = mybir.AluOpType

    groups = _groups_for(rpc)
    n_groups = len(groups)
    s_cols = n_groups * SEGS_PER_BLK

    nc = bacc.Bacc("TRN2", target_bir_lowering=False, debug=False)

    # ---- I/O ----
    xT = nc.dram_tensor("xT", [D, rpc], dt.float32r, kind="ExternalInput")
    w1qT_d = nc.dram_tensor("w1qT", [D, H], dt.float32r, kind="ExternalInput")
    w1wT_d = nc.dram_tensor("w1wT", [D, H], dt.float32r, kind="ExternalInput")
    qb1_d = nc.dram_tensor("qb1", [128, 2], dt.float32, kind="ExternalInput")
    wb1_d = nc.dram_tensor("wb1", [128, 2], dt.float32, kind="ExternalInput")
    # mm2 lhsT variants: variant j has its 16 (or 32) live columns at
    # partition-strip [32j, 32j+32) and zeros elsewhere, so four blocks'
    # mm2 matmuls can accumulate into one shared [128, BLK] psum tile with
    # block j's outputs landing on partitions [32j, 32j+32).
    w2qP_d = nc.dram_tensor("w2qP", [H, 512], dt.bfloat16, kind="ExternalInput")
    ww2P_d = nc.dram_tensor("ww2P", [H, 512], dt.bfloat16, kind="ExternalInput")
    qb2_d = nc.dram_tensor("qb2", [128, 1], dt.float32, kind="ExternalInput")
    wb2_d = nc.dram_tensor("wb2", [128, 1], dt.float32, kind="ExternalInput")
    s_out = nc.dram_tensor("S_out", [128, s_cols], dt.float32, kind="ExternalOutput")

    with tile.TileContext(nc) as tc:
        with (
            tc.tile_pool(name="wpool", bufs=1) as wpool,
            tc.tile_pool(name="xpool", bufs=6) as xpool,
            tc.tile_pool(name="hq", bufs=10) as hq_pool,
            tc.tile_pool(name="hw", bufs=10) as hw_pool,
            tc.tile_pool(name="misc", bufs=3) as misc_pool,
            tc.tile_pool(name="sall", bufs=1) as sall_pool,
            tc.tile_pool(name="ps1", bufs=4, space=bass.MemorySpace.PSUM) as ps1,
            tc.tile_pool(name="ps2", bufs=2, space=bass.MemorySpace.PSUM) as ps2,
        ):
            # ---- load weights/constants into SBUF ----
            w1q_sb = []
            w1w_sb = []
            w2q_sb = []
            ww2_sb = []
            for kc in range(2):
                t = wpool.tile([128, H], dt.float32r, tag=f"w1q{kc}")
                nc.sync.dma_start(t[:, :], xT_sl(w1qT_d, kc))
                w1q_sb.append(t)
                t = wpool.tile([128, H], dt.float32r, tag=f"w1w{kc}")
                nc.sync.dma_start(t[:, :], xT_sl(w1wT_d, kc))
                w1w_sb.append(t)
                w2q_row = []
                ww2_row = []
                for j in range(4):
                    t = wpool.tile([128, 128], dt.bfloat16, tag=f"w2q{kc}_{j}",
                                   name=f"w2q{kc}_{j}")
                    nc.sync.dma_start(
                        t[:, :], w2qP_d[kc * 128:(kc + 1) * 128,
                                        128 * j:128 * (j + 1)])
                    w2q_row.append(t)
                    t = wpool.tile([128, 128], dt.bfloat16, tag=f"ww2{kc}_{j}",
                                   name=f"ww2{kc}_{j}")
                    nc.sync.dma_start(
                        t[:, :], ww2P_d[kc * 128:(kc + 1) * 128,
                                        128 * j:128 * (j + 1)])
                    ww2_row.append(t)
                w2q_sb.append(w2q_row)
                ww2_sb.append(ww2_row)
            qb1_sb = wpool.tile([128, 2], dt.float32, tag="qb1")
            nc.sync.dma_start(qb1_sb[:, :], qb1_d[:, :])
            wb1_sb = wpool.tile([128, 2], dt.float32, tag="wb1")
            nc.sync.dma_start(wb1_sb[:, :], wb1_d[:, :])
            qb2_sb = wpool.tile([128, 1], dt.float32, tag="qb2")
            nc.sync.dma_start(qb2_sb[:, :], qb2_d[:, :])
            wb2_sb = wpool.tile([128, 1], dt.float32, tag="wb2")
            nc.sync.dma_start(wb2_sb[:, :], wb2_d[:, :])

            s_all = sall_pool.tile([128, s_cols], dt.float32, tag="sall")
            nc.vector.memset(s_all[:, :], 0.0)

            for g, nb in groups:
                psum_q2 = ps2.tile([128, BLK], dt.float32, tag="q2")
                psum_sig = ps2.tile([128, BLK], dt.float32, tag="sig")
                for j in range(nb):
                    blk = g * BLKS_PER_GROUP + j
                    c0 = blk * BLK
                    # load x.T block (feature-major), 2 partition chunks
                    xt = [xpool.tile([128, BLK], dt.float32r, tag="x",
                                     name=f"x{blk}_{kc2}") for kc2 in range(2)]
                    for kc in range(2):
                        nc.sync.dma_start(
                            xt[kc][:, :], xT[kc * 128:(kc + 1) * 128, c0:c0 + BLK]
                        )
                    qh = []
                    wh = []
                    for hc in range(2):
                        # quality branch mm1 + ReLU(+qb1) via ACT
                        psq = ps1.tile([128, BLK], dt.float32, tag="mm1")
                        for kc in range(2):
                            nc.tensor.matmul(
                                psq[:, :],
                                w1q_sb[kc][:, hc * 128:(hc + 1) * 128],
                                xt[kc][:, :],
                                start=(kc == 0),
                                stop=(kc == 1),
                            )
                        qh_t = hq_pool.tile([128, BLK], dt.bfloat16, tag="qh")
                        nc.scalar.activation(
                            qh_t[:, :], psq[:, :], Act.Relu,
                            bias=qb1_sb[:, hc:hc + 1], scale=1.0,
                        )
                        qh.append(qh_t)
                        # weight branch mm1 (BN folded) + ReLU(+wb1') via DVE
                        psw = ps1.tile([128, BLK], dt.float32, tag="mm1")
                        for kc in range(2):
                            nc.tensor.matmul(
                                psw[:, :],
                                w1w_sb[kc][:, hc * 128:(hc + 1) * 128],
                                xt[kc][:, :],
                                start=(kc == 0),
                                stop=(kc == 1),
                            )
                        wh_t = hw_pool.tile([128, BLK], dt.bfloat16, tag="wh")
                        nc.vector.tensor_scalar(
                            wh_t[:, :], psw[:, :],
                            wb1_sb[:, hc:hc + 1], 0.0,
                            op0=Alu.add, op1=Alu.max,
                        )
                        wh.append(wh_t)
                    # mm2: zero-padded lhsT variants accumulate block j's
                    # outputs onto partitions [32j, 32j+32) of shared psum
                    for kc in range(2):
                        nc.tensor.matmul(
                            psum_q2[:, :], w2q_sb[kc][j][:, :], qh[kc][:, :],
                            start=(j == 0 and kc == 0),
                            stop=(j == nb - 1 and kc == 1),
                        )
                    for kc in range(2):
                        nc.tensor.matmul(
                            psum_sig[:, :], ww2_sb[kc][j][:, :], wh[kc][:, :],
                            start=(j == 0 and kc == 0),
                            stop=(j == nb - 1 and kc == 1),
                        )
                p = 128
                # w = sigmoid(colsum + wb2), replicated over each 32-row strip
                w_t = misc_pool.tile([128, BLK], dt.bfloat16, tag="w")
                nc.scalar.activation(
                    w_t[:p, :], psum_sig[:p, :], Act.Sigmoid,
                    bias=wb2_sb[:p, 0:1], scale=1.0,
                )
                # t2 = (q2 + qb2) * w
                t2 = misc_pool.tile([128, BLK], dt.bfloat16, tag="t2")
                nc.vector.scalar_tensor_tensor(
                    t2[:p, :], psum_q2[:p, :], qb2_sb[:p, 0:1], w_t[:p, :],
                    op0=Alu.add, op1=Alu.mult,
                )
                # segment sums: [p, 10, 50] -> [p, 10]
                c = g * SEGS_PER_BLK
                nc.vector.tensor_reduce(
                    s_all[:p, c:c + SEGS_PER_BLK],
                    t2[:p, :].rearrange("p (s i) -> p s i", i=SEG),
                    axis=mybir.AxisListType.X,
                    op=Alu.add,
                )

            nc.sync.dma_start(s_out[:, :], s_all[:, :])

    nc.compile()
    return nc


def xT_sl(dram, kc):
    """Rows [kc*128, (kc+1)*128) of a DRAM tensor."""
    return dram[kc * 128:(kc + 1) * 128, :]


def _host_weights(qw1, qb1, qw2, qb2, bn_gamma, bn_beta, bn_mean, bn_var,
                  ww1, wb1, ww2, wb2):
    """Fold BN, transpose weights into lhsT layouts, pack biases."""
    import ml_dtypes

    f32 = np.float32
    s = (bn_gamma / np.sqrt(bn_var + EPS)).astype(f32)
    bpp = (bn_beta - bn_mean * s).astype(f32)
    w1qT = np.ascontiguousarray(qw1.T.astype(f32))                   # [D,H]
    w1wT = np.ascontiguousarray((ww1 * s[None, :]).T.astype(f32))    # [D,H]
    b1w = (ww1.astype(np.float64) @ bpp.astype(np.float64)).astype(f32) + wb1.astype(f32)

    qb1_p = np.ascontiguousarray(qb1.astype(f32).reshape(2, 128).T)  # [128,2]
    wb1_p = np.ascontiguousarray(b1w.reshape(2, 128).T)              # [128,2]

    w2qP = np.zeros((H, 512), dtype=ml_dtypes.bfloat16)
    ww2P = np.zeros((H, 512), dtype=ml_dtypes.bfloat16)
    for j in range(4):
        w2qP[:, 128 * j + 32 * j:128 * j + 32 * j + BINS] = \
            qw2.T.astype(ml_dtypes.bfloat16)
        ww2P[:, 128 * j + 32 * j:128 * j + 32 * j + 32] = \
            np.repeat(ww2.reshape(H, 1).astype(ml_dtypes.bfloat16), 32, axis=1)

    qb2_p = np.zeros((128, 1), dtype=f32)
    for j in range(4):
        qb2_p[32 * j:32 * j + BINS, 0] = qb2.astype(f32)
    wb2_p = np.full((128, 1), np.float32(wb2.reshape(-1)[0]), dtype=f32)

    return dict(w1qT=w1qT, w1wT=w1wT, qb1=qb1_p, wb1=wb1_p,
                w2qP=w2qP, ww2P=ww2P, qb2=qb2_p, wb2=wb2_p)


def _unpack_sums(s_packed, rpc):
    """[128, s_cols] packed -> [rpc//SEG, BINS] segment sums."""
    n_groups = len(_groups_for(rpc))
    segs = rpc // SEG
    arr = s_packed.reshape(4, 32, n_groups, SEGS_PER_BLK)   # [j, b, g, i]
    out = arr.transpose(2, 0, 3, 1).reshape(n_groups * 4 * SEGS_PER_BLK, 32)
    return out[:segs, :BINS]


def _numpy_fallback(data, qw1, qb1, qw2, qb2, bn_gamma, bn_beta, bn_mean,
                    bn_var, ww1, wb1, ww2, wb2, tag, separates):
    """Pure-host reference path (used only for non-uniform segments)."""
    f = np.float32
    x = data.astype(f)
    s = (bn_gamma / np.sqrt(bn_var + EPS)).astype(f)
    xn = (x - bn_mean.astype(f)) * s + bn_beta.astype(f)
    wh = np.maximum(xn @ ww1.T.astype(f) + wb1.astype(f), 0)
    wout = 1.0 / (1.0 + np.exp(-(wh @ ww2.T.astype(f) + wb2.astype(f))))
    qh = np.maximum(x @ qw1.T.astype(f) + qb1.astype(f), 0)
    q = (qh @ qw2.T.astype(f) + qb2.astype(f)) * wout
    sep = np.asarray(separates).astype(np.int64)
    sums = np.add.reduceat(q, sep[:-1], axis=0)
    lengths = (sep[1:] - sep[:-1]).astype(np.float64)
    return sums, lengths


def _finish(sums, lengths, tag):
    """sums [T,BINS], lengths [T] -> (out_probs f32 [T,BINS], loss f32)."""
    logits = sums.astype(np.float64) / lengths[:, None]
    m = logits.max(axis=1, keepdims=True)
    e = np.exp(logits - m)
    z = e.sum(axis=1, keepdims=True)
    probs = e / z
    logp = (logits - m) - np.log(z)
    t = np.asarray(tag).astype(np.int64).reshape(-1)
    loss = -np.mean(logp[np.arange(logp.shape[0]), t])
    return probs.astype(np.float32), np.float32(loss)


def _run_device(data, shared, trace=False):
    from concourse.bass_utils import run_bass_kernel_spmd

    if RPC not in _PROGRAM_CACHE:
        _PROGRAM_CACHE[RPC] = _build_program(RPC)
    nc = _PROGRAM_CACHE[RPC]

    in_maps = []
    for i in range(N_CORES):
        shard = data[i * RPC:(i + 1) * RPC]
        xT = np.ascontiguousarray(shard.T.astype(np.float32))
        m = {"xT": xT}
        m.update(shared)
        in_maps.append(m)

    res = run_bass_kernel_spmd(nc, in_maps, list(range(N_CORES)), trace=trace)
    sums = np.concatenate(
        [_unpack_sums(res.results[i]["S_out"].astype(np.float64), RPC)
         for i in range(N_CORES)], axis=0)
    return sums, res


def _bench_device(data, shared, iters=5):
    """Steady-state timing of the NEFF execution via PJRT.

    Mirrors bass2jax.run_bass_via_pjrt but keeps the jitted callable and
    device-resident inputs so repeated calls time only the on-device
    execution (plus PJRT dispatch). Returns (results_list, times_sec).
    """
    import time

    import jax
    from jax.experimental.shard_map import shard_map
    from jax.sharding import Mesh, PartitionSpec

    from concourse import bass2jax, mybir

    if RPC not in _PROGRAM_CACHE:
        _PROGRAM_CACHE[RPC] = _build_program(RPC)
    nc = _PROGRAM_CACHE[RPC]
    bass2jax.install_neuronx_cc_hook()

    in_maps = []
    for i in range(N_CORES):
        shard = data[i * RPC:(i + 1) * RPC]
        xT = np.ascontiguousarray(shard.T.astype(np.float32))
        m = {"xT": xT}
        m.update(shared)
        in_maps.append(m)

    partition_name = (nc.partition_id_tensor.name
                      if nc.partition_id_tensor else None)
    in_names, out_names, out_avals, zero_outs = [], [], [], []
    for alloc in nc.m.functions[0].allocations:
        if not isinstance(alloc, mybir.MemoryLocationSet):
            continue
        name = alloc.memorylocations[0].name
        if alloc.kind == "ExternalInput":
            if name != partition_name:
                in_names.append(name)
        elif alloc.kind == "ExternalOutput":
            shape = tuple(alloc.tensor_shape)
            dtype = mybir.dt.np(alloc.dtype)
            out_names.append(name)
            out_avals.append(jax.core.ShapedArray(shape, dtype))
            zero_outs.append(np.zeros(shape, dtype))
    n_params = len(in_names)
    n_outs = len(out_avals)
    all_names = in_names + out_names
    if partition_name is not None:
        all_names = all_names + [partition_name]

    def _body(*args):
        operands = list(args)
        if partition_name is not None:
            operands.append(bass2jax.partition_id_tensor())
        outs = bass2jax._bass_exec_p.bind(
            *operands,
            out_avals=tuple(out_avals),
            in_names=tuple(all_names),
            out_names=tuple(out_names),
            lowering_input_output_aliases=(),
            sim_require_finite=True,
            sim_require_nnan=True,
            nc=nc,
        )
        return tuple(outs)

    devices = jax.devices()[:N_CORES]
    mesh = Mesh(np.asarray(devices), ("core",))
    in_specs = (PartitionSpec("core"),) * (n_params + n_outs)
    out_specs = (PartitionSpec("core"),) * n_outs
    sharded = jax.jit(
        shard_map(_body, mesh=mesh, in_specs=in_specs, out_specs=out_specs,
                  check_rep=False),
        keep_unused=True,
    )
    concat_in = [
        np.concatenate([np.asarray(in_maps[c][nm]) for c in range(N_CORES)], axis=0)
        for nm in in_names
    ]
    concat_zeros = [
        np.zeros((N_CORES * z.shape[0], *z.shape[1:]), z.dtype) for z in zero_outs
    ]
    shard = jax.sharding.NamedSharding(mesh, PartitionSpec("core"))
    dev_in = [jax.device_put(a, shard) for a in concat_in]
    dev_zeros = [jax.device_put(a, shard) for a in concat_zeros]

    out_arrs = sharded(*dev_in, *dev_zeros)  # warm-up + result
    jax.block_until_ready(out_arrs)
    times = []
    for _ in range(iters):
        t0 = time.perf_counter()
        o = sharded(*dev_in, *dev_zeros)
        jax.block_until_ready(o)
        times.append(time.perf_counter() - t0)

    results = [
        {nm: np.asarray(out_arrs[i]).reshape(N_CORES, *out_avals[i].shape)[c]
         for i, nm in enumerate(out_names)}
        for c in range(N_CORES)
    ]
    return results, times


def kernel(data, qw1, qb1, qw2, qb2, bn_gamma, bn_beta, bn_mean, bn_var,
           ww1, wb1, ww2, wb2, tag, separates, _trace=False, _return_res=False):
    data = np.asarray(data)
    sep = np.asarray(separates).astype(np.int64)
    args = [np.asarray(a) for a in (qw1, qb1, qw2, qb2, bn_gamma, bn_beta,
                                    bn_mean, bn_var, ww1, wb1, ww2, wb2)]
    uniform = (
        data.shape == (N_TOTAL, D)
        and sep.shape[0] == T_SEGS + 1
        and np.all(np.diff(sep) == SEG)
        and sep[0] == 0
    )
    if not uniform:
        sums, lengths = _numpy_fallback(data, *args, tag, sep)
        return _finish(sums, lengths, tag)

    shared = _host_weights(*args)
    sums, res = _run_device(data, shared, trace=_trace)
    lengths = np.full(T_SEGS, float(SEG))
    out = _finish(sums, lengths, tag)
    if _return_res:
        return out, res
    return out


# revision 15
# speedup vs baseline: 5395.6452x; 3473.7873x over previous
"""Trainium2 Bass kernel for nn_DoubleNet (segment_reduce).

Computation (per reference):
  weight branch : BN(eval) -> Linear(D,H) -> ReLU -> Linear(H,1) -> Sigmoid   [N,1]
  quality branch: Linear(D,H) -> ReLU -> Linear(H,BINS), scaled by weight     [N,BINS]
  segment mean over uniform 50-row segments -> [T,BINS] -> softmax / NLL loss

Device strategy (8 cores, data-parallel over rows):
  - Host pre-transposes each core's data shard to feature-major xT [D, rows]
    and folds the BatchNorm affine into the weight-branch Linear.
  - Feature-major pipeline per 500-row block:
      mm1 (f32r matmuls, K=256 in 2 chunks)  -> psum [128,500] per h-chunk
      ReLU+bias on the psum->sbuf copy (ACT for quality, DVE for weight branch)
      mm2-q: lhsT = qw2.T (padded M=32), col-tiled 4 blocks per psum tile
      mm2-w: lhsT = ww2 replicated across M -> column-sum trick: psum gets the
             sigmoid argument replicated; ACT applies Sigmoid(+wb2)
      DVE: (q2 + qb2) * w  fused via scalar_tensor_tensor, then a strided
           segment reduce [128,10,50] -> [128,10] into a packed accumulator.
  - Device output per core: packed segment sums [128, 320] (f32).
  - Host: unpack, /50, softmax, loss (tiny [T,16] math).
"""

import os
import sys

sys.path.insert(0, "/opt/trn_rl_repo")

import numpy as np  # noqa: E402

N_TOTAL = 500000
D = 256
H = 256
BINS = 16
T_SEGS = 10000
SEG = 50
EPS = 1e-5

N_CORES = 8
RPC = N_TOTAL // N_CORES          # 62500 rows per core
BLK = 500                         # rows per matmul block (10 segments)
SEGS_PER_BLK = BLK // SEG         # 10
BLKS_PER_GROUP = 4                # packed into one psum tile via col-tiling

_PROGRAM_CACHE = {}


def _groups_for(rpc):
    """List of (group_index, n_blocks) covering rpc rows."""
    nblk = rpc // BLK
    assert nblk * BLK == rpc
    groups = []
    g = 0
    rem = nblk
    while rem > 0:
        nb = min(BLKS_PER_GROUP, rem)
        groups.append((g, nb))
        g += 1
        rem -= nb
    return groups


def _build_program(rpc, reps=1):
    """Build the Bass program for one core processing `rpc` rows.

    reps > 1 repeats the whole pipeline (identical work) inside the NEFF —
    used to measure per-iteration device time by slope, since the axon
    PJRT dispatch has ~90 ms of fixed overhead that hides kernel time.
    """
    import concourse.bass as bass
    import concourse.tile as tile
    from concourse import bacc, mybir

    dt = mybir.dt
    Act = mybir.ActivationFunctionType
    Alu = mybir.AluOpType

    groups = _groups_for(rpc)
    n_groups = len(groups)
    s_cols = n_groups * SEGS_PER_BLK

    nc = bacc.Bacc("TRN2", target_bir_lowering=False, debug=False)

    # ---- I/O ----
    xT = nc.dram_tensor("xT", [D, rpc], dt.float32r, kind="ExternalInput")
    w1qT_d = nc.dram_tensor("w1qT", [D, H], dt.float32r, kind="ExternalInput")
    w1wT_d = nc.dram_tensor("w1wT", [D, H], dt.float32r, kind="ExternalInput")
    qb1_d = nc.dram_tensor("qb1", [128, 2], dt.float32, kind="ExternalInput")
    wb1_d = nc.dram_tensor("wb1", [128, 2], dt.float32, kind="ExternalInput")
    # mm2 lhsT variants: variant j has its 16 (or 32) live columns at
    # partition-strip [32j, 32j+32) and zeros elsewhere, so four blocks'
    # mm2 matmuls can accumulate into one shared [128, BLK] psum tile with
    # block j's outputs landing on partitions [32j, 32j+32).
    w2qP_d = nc.dram_tensor("w2qP", [H, 512], dt.bfloat16, kind="ExternalInput")
    ww2P_d = nc.dram_tensor("ww2P", [H, 512], dt.bfloat16, kind="ExternalInput")
    qb2_d = nc.dram_tensor("qb2", [128, 1], dt.float32, kind="ExternalInput")
    wb2_d = nc.dram_tensor("wb2", [128, 1], dt.float32, kind="ExternalInput")
    s_out = nc.dram_tensor("S_out", [128, s_cols], dt.float32, kind="ExternalOutput")

    with tile.TileContext(nc) as tc:
        with (
            tc.tile_pool(name="wpool", bufs=1) as wpool,
            tc.tile_pool(name="xpool", bufs=6) as xpool,
            tc.tile_pool(name="hq", bufs=10) as hq_pool,
            tc.tile_pool(name="hw", bufs=10) as hw_pool,
            tc.tile_pool(name="misc", bufs=3) as misc_pool,
            tc.tile_pool(name="sall", bufs=1) as sall_pool,
            tc.tile_pool(name="ps1", bufs=4, space=bass.MemorySpace.PSUM) as ps1,
            tc.tile_pool(name="ps2", bufs=2, space=bass.MemorySpace.PSUM) as ps2,
        ):
            # ---- load weights/constants into SBUF ----
            w1q_sb = []
            w1w_sb = []
            w2q_sb = []
            ww2_sb = []
            for kc in range(2):
                t = wpool.tile([128, H], dt.float32r, tag=f"w1q{kc}")
                nc.sync.dma_start(t[:, :], xT_sl(w1qT_d, kc))
                w1q_sb.append(t)
                t = wpool.tile([128, H], dt.float32r, tag=f"w1w{kc}")
                nc.sync.dma_start(t[:, :], xT_sl(w1wT_d, kc))
                w1w_sb.append(t)
                w2q_row = []
                ww2_row = []
                for j in range(4):
                    t = wpool.tile([128, 128], dt.bfloat16, tag=f"w2q{kc}_{j}",
                                   name=f"w2q{kc}_{j}")
                    nc.sync.dma_start(
                        t[:, :], w2qP_d[kc * 128:(kc + 1) * 128,
                                        128 * j:128 * (j + 1)])
                    w2q_row.append(t)
                    t = wpool.tile([128, 128], dt.bfloat16, tag=f"ww2{kc}_{j}",
                                   name=f"ww2{kc}_{j}")
                    nc.sync.dma_start(
                        t[:, :], ww2P_d[kc * 128:(kc + 1) * 128,
                                        128 * j:128 * (j + 1)])
                    ww2_row.append(t)
                w2q_sb.append(w2q_row)
                ww2_sb.append(ww2_row)
            qb1_sb = wpool.tile([128, 2], dt.float32, tag="qb1")
            nc.sync.dma_start(qb1_sb[:, :], qb1_d[:, :])
            wb1_sb = wpool.tile([128, 2], dt.float32, tag="wb1")
            nc.sync.dma_start(wb1_sb[:, :], wb1_d[:, :])
            qb2_sb = wpool.tile([128, 1], dt.float32, tag="qb2")
            nc.sync.dma_start(qb2_sb[:, :], qb2_d[:, :])
            wb2_sb = wpool.tile([128, 1], dt.float32, tag="wb2")
            nc.sync.dma_start(wb2_sb[:, :], wb2_d[:, :])

            s_all = sall_pool.tile([128, s_cols], dt.float32, tag="sall")
            nc.vector.memset(s_all[:, :], 0.0)

            for rep in range(reps):
              for g, nb in groups:
                psum_q2 = ps2.tile([128, BLK], dt.float32, tag="q2",
                                   name=f"q2_{rep}_{g}")
                psum_sig = ps2.tile([128, BLK], dt.float32, tag="sig",
                                    name=f"sig_{rep}_{g}")
                for j in range(nb):
                    blk = g * BLKS_PER_GROUP + j
                    c0 = blk * BLK
                    # load x.T block (feature-major), 2 partition chunks
                    xt = [xpool.tile([128, BLK], dt.float32r, tag="x",
                                     name=f"x{blk}_{kc2}") for kc2 in range(2)]
                    for kc in range(2):
                        nc.sync.dma_start(
                            xt[kc][:, :], xT[kc * 128:(kc + 1) * 128, c0:c0 + BLK]
                        )
                    qh = []
                    wh = []
                    for hc in range(2):
                        # quality branch mm1 + ReLU(+qb1) via ACT
                        psq = ps1.tile([128, BLK], dt.float32, tag="mm1")
                        for kc in range(2):
                            nc.tensor.matmul(
                                psq[:, :],
                                w1q_sb[kc][:, hc * 128:(hc + 1) * 128],
                                xt[kc][:, :],
                                start=(kc == 0),
                                stop=(kc == 1),
                            )
                        qh_t = hq_pool.tile([128, BLK], dt.bfloat16, tag="qh")
                        nc.scalar.activation(
                            qh_t[:, :], psq[:, :], Act.Relu,
                            bias=qb1_sb[:, hc:hc + 1], scale=1.0,
                        )
                        qh.append(qh_t)
                        # weight branch mm1 (BN folded) + ReLU(+wb1') via DVE
                        psw = ps1.tile([128, BLK], dt.float32, tag="mm1")
                        for kc in range(2):
                            nc.tensor.matmul(
                                psw[:, :],
                                w1w_sb[kc][:, hc * 128:(hc + 1) * 128],
                                xt[kc][:, :],
                                start=(kc == 0),
                                stop=(kc == 1),
                            )
                        wh_t = hw_pool.tile([128, BLK], dt.bfloat16, tag="wh")
                        nc.vector.tensor_scalar(
                            wh_t[:, :], psw[:, :],
                            wb1_sb[:, hc:hc + 1], 0.0,
                            op0=Alu.add, op1=Alu.max,
                        )
                        wh.append(wh_t)
                    # mm2: zero-padded lhsT variants accumulate block j's
                    # outputs onto partitions [32j, 32j+32) of shared psum
                    for kc in range(2):
                        nc.tensor.matmul(
                            psum_q2[:, :], w2q_sb[kc][j][:, :], qh[kc][:, :],
                            start=(j == 0 and kc == 0),
                            stop=(j == nb - 1 and kc == 1),
                        )
                    for kc in range(2):
                        nc.tensor.matmul(
                            psum_sig[:, :], ww2_sb[kc][j][:, :], wh[kc][:, :],
                            start=(j == 0 and kc == 0),
                            stop=(j == nb - 1 and kc == 1),
                        )
                p = 128
                # w = sigmoid(colsum + wb2), replicated over each 32-row strip
                w_t = misc_pool.tile([128, BLK], dt.bfloat16, tag="w")
                nc.scalar.activation(
                    w_t[:p, :], psum_sig[:p, :], Act.Sigmoid,
                    bias=wb2_sb[:p, 0:1], scale=1.0,
                )
                # t2 = (q2 + qb2) * w
                t2 = misc_pool.tile([128, BLK], dt.bfloat16, tag="t2")
                nc.vector.scalar_tensor_tensor(
                    t2[:p, :], psum_q2[:p, :], qb2_sb[:p, 0:1], w_t[:p, :],
                    op0=Alu.add, op1=Alu.mult,
                )
                # segment sums: [p, 10, 50] -> [p, 10]
                c = g * SEGS_PER_BLK
                nc.vector.tensor_reduce(
                    s_all[:p, c:c + SEGS_PER_BLK],
                    t2[:p, :].rearrange("p (s i) -> p s i", i=SEG),
                    axis=mybir.AxisListType.X,
                    op=Alu.add,
                )

            nc.sync.dma_start(s_out[:, :], s_all[:, :])

    nc.compile()
    return nc


def xT_sl(dram, kc):
    """Rows [kc*128, (kc+1)*128) of a DRAM tensor."""
    return dram[kc * 128:(kc + 1) * 128, :]


def _host_weights(qw1, qb1, qw2, qb2, bn_gamma, bn_beta, bn_mean, bn_var,
                  ww1, wb1, ww2, wb2):
    """Fold BN, transpose weights into lhsT layouts, pack biases."""
    import ml_dtypes

    f32 = np.float32
    s = (bn_gamma / np.sqrt(bn_var + EPS)).astype(f32)
    bpp = (bn_beta - bn_mean * s).astype(f32)
    w1qT = np.ascontiguousarray(qw1.T.astype(f32))                   # [D,H]
    w1wT = np.ascontiguousarray((ww1 * s[None, :]).T.astype(f32))    # [D,H]
    b1w = (ww1.astype(np.float64) @ bpp.astype(np.float64)).astype(f32) + wb1.astype(f32)

    qb1_p = np.ascontiguousarray(qb1.astype(f32).reshape(2, 128).T)  # [128,2]
    wb1_p = np.ascontiguousarray(b1w.reshape(2, 128).T)              # [128,2]

    w2qP = np.zeros((H, 512), dtype=ml_dtypes.bfloat16)
    ww2P = np.zeros((H, 512), dtype=ml_dtypes.bfloat16)
    for j in range(4):
        w2qP[:, 128 * j + 32 * j:128 * j + 32 * j + BINS] = \
            qw2.T.astype(ml_dtypes.bfloat16)
        ww2P[:, 128 * j + 32 * j:128 * j + 32 * j + 32] = \
            np.repeat(ww2.reshape(H, 1).astype(ml_dtypes.bfloat16), 32, axis=1)

    qb2_p = np.zeros((128, 1), dtype=f32)
    for j in range(4):
        qb2_p[32 * j:32 * j + BINS, 0] = qb2.astype(f32)
    wb2_p = np.full((128, 1), np.float32(wb2.reshape(-1)[0]), dtype=f32)

    return dict(w1qT=w1qT, w1wT=w1wT, qb1=qb1_p, wb1=wb1_p,
                w2qP=w2qP, ww2P=ww2P, qb2=qb2_p, wb2=wb2_p)


def _unpack_sums(s_packed, rpc):
    """[128, s_cols] packed -> [rpc//SEG, BINS] segment sums."""
    n_groups = len(_groups_for(rpc))
    segs = rpc // SEG
    arr = s_packed.reshape(4, 32, n_groups, SEGS_PER_BLK)   # [j, b, g, i]
    out = arr.transpose(2, 0, 3, 1).reshape(n_groups * 4 * SEGS_PER_BLK, 32)
    return out[:segs, :BINS]


def _numpy_fallback(data, qw1, qb1, qw2, qb2, bn_gamma, bn_beta, bn_mean,
                    bn_var, ww1, wb1, ww2, wb2, tag, separates):
    """Pure-host reference path (used only for non-uniform segments)."""
    f = np.float32
    x = data.astype(f)
    s = (bn_gamma / np.sqrt(bn_var + EPS)).astype(f)
    xn = (x - bn_mean.astype(f)) * s + bn_beta.astype(f)
    wh = np.maximum(xn @ ww1.T.astype(f) + wb1.astype(f), 0)
    wout = 1.0 / (1.0 + np.exp(-(wh @ ww2.T.astype(f) + wb2.astype(f))))
    qh = np.maximum(x @ qw1.T.astype(f) + qb1.astype(f), 0)
    q = (qh @ qw2.T.astype(f) + qb2.astype(f)) * wout
    sep = np.asarray(separates).astype(np.int64)
    sums = np.add.reduceat(q, sep[:-1], axis=0)
    lengths = (sep[1:] - sep[:-1]).astype(np.float64)
    return sums, lengths


def _finish(sums, lengths, tag):
    """sums [T,BINS], lengths [T] -> (out_probs f32 [T,BINS], loss f32)."""
    logits = sums.astype(np.float64) / lengths[:, None]
    m = logits.max(axis=1, keepdims=True)
    e = np.exp(logits - m)
    z = e.sum(axis=1, keepdims=True)
    probs = e / z
    logp = (logits - m) - np.log(z)
    t = np.asarray(tag).astype(np.int64).reshape(-1)
    loss = -np.mean(logp[np.arange(logp.shape[0]), t])
    return probs.astype(np.float32), np.float32(loss)


def _run_device(data, shared, trace=False):
    from concourse.bass_utils import run_bass_kernel_spmd

    key = (RPC, 1)
    if key not in _PROGRAM_CACHE:
        _PROGRAM_CACHE[key] = _build_program(RPC)
    nc = _PROGRAM_CACHE[key]

    in_maps = []
    for i in range(N_CORES):
        shard = data[i * RPC:(i + 1) * RPC]
        xT = np.ascontiguousarray(shard.T.astype(np.float32))
        m = {"xT": xT}
        m.update(shared)
        in_maps.append(m)

    res = run_bass_kernel_spmd(nc, in_maps, list(range(N_CORES)), trace=trace)
    sums = np.concatenate(
        [_unpack_sums(res.results[i]["S_out"].astype(np.float64), RPC)
         for i in range(N_CORES)], axis=0)
    return sums, res


def _bench_device(data, shared, iters=5, reps=1):
    """Steady-state timing of the NEFF execution via PJRT.

    Mirrors bass2jax.run_bass_via_pjrt but keeps the jitted callable and
    device-resident inputs so repeated calls time only the on-device
    execution (plus PJRT dispatch). Returns (results_list, times_sec).
    """
    import time

    import jax
    from jax.experimental.shard_map import shard_map
    from jax.sharding import Mesh, PartitionSpec

    from concourse import bass2jax, mybir

    key = (RPC, reps)
    if key not in _PROGRAM_CACHE:
        _PROGRAM_CACHE[key] = _build_program(RPC, reps=reps)
    nc = _PROGRAM_CACHE[key]
    bass2jax.install_neuronx_cc_hook()

    in_maps = []
    for i in range(N_CORES):
        shard = data[i * RPC:(i + 1) * RPC]
        xT = np.ascontiguousarray(shard.T.astype(np.float32))
        m = {"xT": xT}
        m.update(shared)
        in_maps.append(m)

    partition_name = (nc.partition_id_tensor.name
                      if nc.partition_id_tensor else None)
    in_names, out_names, out_avals, zero_outs = [], [], [], []
    for alloc in nc.m.functions[0].allocations:
        if not isinstance(alloc, mybir.MemoryLocationSet):
            continue
        name = alloc.memorylocations[0].name
        if alloc.kind == "ExternalInput":
            if name != partition_name:
                in_names.append(name)
        elif alloc.kind == "ExternalOutput":
            shape = tuple(alloc.tensor_shape)
            dtype = mybir.dt.np(alloc.dtype)
            out_names.append(name)
            out_avals.append(jax.core.ShapedArray(shape, dtype))
            zero_outs.append(np.zeros(shape, dtype))
    n_params = len(in_names)
    n_outs = len(out_avals)
    all_names = in_names + out_names
    if partition_name is not None:
        all_names = all_names + [partition_name]

    def _body(*args):
        operands = list(args)
        if partition_name is not None:
            operands.append(bass2jax.partition_id_tensor())
        outs = bass2jax._bass_exec_p.bind(
            *operands,
            out_avals=tuple(out_avals),
            in_names=tuple(all_names),
            out_names=tuple(out_names),
            lowering_input_output_aliases=(),
            sim_require_finite=True,
            sim_require_nnan=True,
            nc=nc,
        )
        return tuple(outs)

    devices = jax.devices()[:N_CORES]
    mesh = Mesh(np.asarray(devices), ("core",))
    in_specs = (PartitionSpec("core"),) * (n_params + n_outs)
    out_specs = (PartitionSpec("core"),) * n_outs
    sharded = jax.jit(
        shard_map(_body, mesh=mesh, in_specs=in_specs, out_specs=out_specs,
                  check_rep=False),
        keep_unused=True,
    )
    concat_in = [
        np.concatenate([np.asarray(in_maps[c][nm]) for c in range(N_CORES)], axis=0)
        for nm in in_names
    ]
    concat_zeros = [
        np.zeros((N_CORES * z.shape[0], *z.shape[1:]), z.dtype) for z in zero_outs
    ]
    shard = jax.sharding.NamedSharding(mesh, PartitionSpec("core"))
    dev_in = [jax.device_put(a, shard) for a in concat_in]
    dev_zeros = [jax.device_put(a, shard) for a in concat_zeros]

    out_arrs = sharded(*dev_in, *dev_zeros)  # warm-up + result
    jax.block_until_ready(out_arrs)
    times = []
    for _ in range(iters):
        t0 = time.perf_counter()
        o = sharded(*dev_in, *dev_zeros)
        jax.block_until_ready(o)
        times.append(time.perf_counter() - t0)

    results = [
        {nm: np.asarray(out_arrs[i]).reshape(N_CORES, *out_avals[i].shape)[c]
         for i, nm in enumerate(out_names)}
        for c in range(N_CORES)
    ]
    return results, times


def kernel(data, qw1, qb1, qw2, qb2, bn_gamma, bn_beta, bn_mean, bn_var,
           ww1, wb1, ww2, wb2, tag, separates, _trace=False, _return_res=False):
    data = np.asarray(data)
    sep = np.asarray(separates).astype(np.int64)
    args = [np.asarray(a) for a in (qw1, qb1, qw2, qb2, bn_gamma, bn_beta,
                                    bn_mean, bn_var, ww1, wb1, ww2, wb2)]
    uniform = (
        data.shape == (N_TOTAL, D)
        and sep.shape[0] == T_SEGS + 1
        and np.all(np.diff(sep) == SEG)
        and sep[0] == 0
    )
    if not uniform:
        sums, lengths = _numpy_fallback(data, *args, tag, sep)
        return _finish(sums, lengths, tag)

    shared = _host_weights(*args)
    sums, res = _run_device(data, shared, trace=_trace)
    lengths = np.full(T_SEGS, float(SEG))
    out = _finish(sums, lengths, tag)
    if _return_res:
        return out, res
    return out
